# revision 3
# baseline (speedup 1.0000x reference)
"""Trainium2 Bass kernel for the DLI loss (ragged segment means -> pairwise NLL).

Math reduction used here
------------------------
reference() computes, per batch b:
  states[t] = mean of encoder_output[b, start_t:end_t+1, :]        (ragged turns)
  logits[j,k] = cat(states[j], states[k]) @ W + bias               ([T,T,2])
  loss = mean over pairs k<j of NLL(log_softmax(logits), target=(k==j-1))

With 2 classes only the logit difference matters:
  u[j,k] = A[j] + C[k] + (bias[1]-bias[0])
  A[j] = states[j] . (Wl[:,1]-Wl[:,0]),  C[k] = states[k] . (Wr[:,1]-Wr[:,0])
  nll  = softplus(u) for target 0, softplus(-u) for target 1.

So the only heavy work is the ragged segment SUM of encoder_output (256 MB read),
expressed as a masked matmul: seg[T,D] = M[S,T]^T @ x[S,D] with M a 0/1 segment
membership mask built on-device from iota/compare against the turn end ids.
Each core handles 4 of the 32 batches (pure data parallel).

Pipeline shape (from trace analysis): the kernel is HBM-DMA-bound (~410 GB/s
sustained). x streams in 1 MB pieces (4 chunks of [128, 512]) so the
f32->bf16 cast + matmul chase the DMA closely instead of lagging a 4 MB
tile behind. Batches 0-2 finish with on-device dots (seg . wl, seg . wr);
batch 3 ships its raw segment-sum psum to the host so the final-tail serial
DVE chain (mul+reduce x2, ~2.7us) is replaced by a single copy + 128 KB DMA.
The host finishes the tiny [T,T] softplus triangle in f64.

bf16 is used for the matmul operands (mask is exact 0/1; x rounds to ~0.2%
per element). The final loss averages 64512 pairs, so the bf16 noise washes
out to ~3e-7 relative error on the scalar output (measured).
"""

import sys
import os

sys.path.insert(0, "/opt/trn_rl_repo")

# The bass kernel executes through PJRT on the axon-tunneled NeuronCores; if a
# caller pinned JAX_PLATFORMS to something without axon (and jax isn't imported
# yet), undo that so jax.devices() can see the 8 cores.
_jp = os.environ.get("JAX_PLATFORMS")
if _jp is not None and "axon" not in _jp and "jax" not in sys.modules:
    del os.environ["JAX_PLATFORMS"]

import numpy as np

# Problem shapes (hardcoded per harness contract).
B, S, D, T = 32, 4096, 512, 64
N_CORES = 8
BPC = B // N_CORES          # batches per core
P = 128                     # SBUF partitions
NCH = S // P                # 32 chunks of [128, D] per batch
PC = 4                      # chunks per DMA piece (1 MB f32)
NP = NCH // PC              # 8 pieces per batch
# Position mapping within piece q: s = 512*q + PC*p + c. Each partition reads
# one contiguous PC*D*4 = 8 KB block per DMA.

_PROGRAM_CACHE = {}


def _build_program():
    """Build + compile the per-core Bass/Tile program (identical on all cores)."""
    from contextlib import ExitStack

    import concourse.bacc as bacc
    import concourse.mybir as mybir
    import concourse.tile as tile

    f32 = mybir.dt.float32
    bf16 = mybir.dt.bfloat16

    nc = bacc.Bacc(
        "TRN2", target_bir_lowering=False, debug=False, enable_asserts=False
    )

    x_d = nc.dram_tensor("x", [BPC, S, D], f32, kind="ExternalInput").ap()
    ends_d = nc.dram_tensor("endsb", [BPC, T], f32, kind="ExternalInput").ap()
    wlr_d = nc.dram_tensor("wlr", [2, D], f32, kind="ExternalInput").ap()
    dots_d = nc.dram_tensor("dots", [T, BPC - 1, 2], f32, kind="ExternalOutput").ap()
    seg3_d = nc.dram_tensor("seg3", [T, D], f32, kind="ExternalOutput").ap()

    with tile.TileContext(nc) as tc, ExitStack() as ctx:
        singles = ctx.enter_context(tc.tile_pool(name="singles", bufs=1))
        xpool = ctx.enter_context(tc.tile_pool(name="xp", bufs=6))
        bpool = ctx.enter_context(tc.tile_pool(name="bp", bufs=4))
        mpool = ctx.enter_context(tc.tile_pool(name="mp", bufs=2))
        epool = ctx.enter_context(tc.tile_pool(name="ep", bufs=4))
        spool = ctx.enter_context(tc.tile_pool(name="sp", bufs=2))
        ppool = ctx.enter_context(tc.tile_pool(name="pp", bufs=2, space="PSUM"))

        dma_plan = [(b, q) for b in range(BPC) for q in range(NP)]

        def x_dma(b, q):
            xt = xpool.tile([P, PC, D], f32, tag="xt")
            nc.sync.dma_start(
                xt[:],
                x_d[b][512 * q : 512 * (q + 1), :].rearrange(
                    "(p c) d -> p c d", c=PC
                ),
            )
            return xt

        # Prefetch the first pieces before any setup work so the DMA queue
        # never drains while the tiny ends/wlr transfers trigger.
        from collections import deque

        PREFETCH = 3
        xq = deque(x_dma(*dma_plan[i]) for i in range(PREFETCH))
        next_dma = PREFETCH

        # ends on every partition, for all batches upfront (tiny).
        ends_ts = []
        for b in range(BPC):
            et = epool.tile([P, 1, T], f32, tag=f"ends{b}")
            nc.sync.dma_start(
                et[:], ends_d[b].unsqueeze(0).unsqueeze(0).to_broadcast((P, 1, T))
            )
            ends_ts.append(et)

        # wl/wr difference vectors replicated on T partitions for the row dots.
        wlr_t = singles.tile([T, 2, D], f32)
        nc.sync.dma_start(wlr_t[:], wlr_d.unsqueeze(0).to_broadcast((T, 2, D)))

        # Position index table: piece q holds s = 512*q + PC*p + c at [p, 4q+c].
        iota_t = singles.tile([P, NCH, T], f32, tag="iota_t")
        for q in range(NP):
            nc.gpsimd.iota(
                iota_t[:, PC * q : PC * (q + 1), :],
                [[1, PC], [0, T]],
                base=512 * q,
                channel_multiplier=PC,
                allow_small_or_imprecise_dtypes=True,
            )

        dots_t = singles.tile([T, BPC - 1, 2], f32)
        seg3_t = singles.tile([T, D], f32)

        for b in range(BPC):
            # mask[p,i,t] = (s <= end_t) - (s <= end_{t-1}) in {0,1}, bf16.
            cmpe = mpool.tile([P, NCH, T], bf16, tag="cmpe")
            mask = mpool.tile([P, NCH, T], bf16, tag="mask")
            nc.vector.tensor_tensor(
                cmpe[:],
                iota_t[:],
                ends_ts[b][:].to_broadcast((P, NCH, T)),
                op=mybir.AluOpType.is_le,
            )
            nc.vector.tensor_sub(
                mask[:, :, 1:], cmpe[:, :, 1:], cmpe[:, :, : T - 1]
            )
            nc.vector.tensor_copy(mask[:, :, 0:1], cmpe[:, :, 0:1])

            psum = ppool.tile([T, D], f32)
            for q in range(NP):
                xt = xq.popleft()
                if next_dma < len(dma_plan):
                    xq.append(x_dma(*dma_plan[next_dma]))
                    next_dma += 1
                xb = bpool.tile([P, PC, D], bf16, tag="xb")
                # Alternate the cast between ACT and DVE so neither engine
                # becomes the pipeline gate; the final piece casts on the
                # faster DVE.
                last_piece = b == BPC - 1 and q == NP - 1
                use_act = (q % 2 == 0) and not last_piece
                eng = nc.scalar.copy if use_act else nc.vector.tensor_copy
                eng(xb[:], xt[:])
                for c in range(PC):
                    i = PC * q + c
                    nc.tensor.matmul(
                        psum[:],
                        mask[:, i, :],
                        xb[:, c, :],
                        start=(i == 0),
                        stop=(i == NCH - 1),
                    )

            if b < BPC - 1:
                # A0[j] = sum_d seg[j,d]*wl[d] ; C0[j] = sum_d seg[j,d]*wr[d]
                for d_ in range(2):
                    scratch = spool.tile([T, D], f32, tag=f"scr{d_}")
                    nc.vector.tensor_mul(scratch[:], psum[:], wlr_t[:, d_, :])
                    nc.vector.reduce_sum(
                        dots_t[:, b, d_ : d_ + 1],
                        scratch[:],
                        axis=mybir.AxisListType.X,
                    )
                if b == BPC - 2:
                    # All on-device dots done; ship them mid-stream.
                    nc.sync.dma_start(dots_d[:], dots_t[:])
            else:
                # Last batch: ship the raw segment sums; host applies wl/wr.
                nc.vector.tensor_copy(seg3_t[:], psum[:])
                nc.sync.dma_start(seg3_d[:], seg3_t[:])

    nc.compile()
    return nc


def _host_prep(encoder_output, W, b, his_turn_end_ids):
    x = np.ascontiguousarray(np.asarray(encoder_output, dtype=np.float32))
    W = np.asarray(W, dtype=np.float32)
    bias = np.asarray(b, dtype=np.float32)
    ends = np.asarray(his_turn_end_ids).astype(np.int64)

    ends_prev = np.concatenate(
        [np.full((B, 1), -1, np.int64), ends[:, :-1]], axis=1
    )
    endsb = ends.astype(np.float32)  # [B, T]

    wlr = np.stack([W[:D, 1] - W[:D, 0], W[D:, 1] - W[D:, 0]], axis=0)  # [2, D]
    wlr = np.ascontiguousarray(wlr, dtype=np.float32)
    bd = np.float64(np.float32(bias[1]) - np.float32(bias[0]))

    counts = (ends - ends_prev).astype(np.float64)  # [B, T]
    return x, endsb, wlr, bd, counts


def _host_finish(A0, C0, counts, bd):
    """A0/C0: [B, T] raw dots of segment sums; returns the scalar loss."""
    A = A0.astype(np.float64) / counts
    C = C0.astype(np.float64) / counts
    u = A[:, :, None] + C[:, None, :] + bd  # [B, T, T]
    j = np.arange(T)[:, None]
    k = np.arange(T)[None, :]
    tri = k < j
    adj = k == (j - 1)
    nll = np.where(adj, np.logaddexp(0.0, -u), np.logaddexp(0.0, u))
    n_pairs = B * (T * (T - 1) // 2)
    loss = np.sum(np.where(tri, nll, 0.0)) / n_pairs
    return np.asarray(loss, dtype=np.float32)


def kernel(encoder_output, W, b, his_turn_end_ids):
    from concourse.bass_utils import run_bass_kernel_spmd

    x, endsb, wlr, bd, counts = _host_prep(encoder_output, W, b, his_turn_end_ids)

    if "nc" not in _PROGRAM_CACHE:
        _PROGRAM_CACHE["nc"] = _build_program()
    nc = _PROGRAM_CACHE["nc"]

    in_maps = [
        {
            "x": x[i * BPC : (i + 1) * BPC],
            "endsb": endsb[i * BPC : (i + 1) * BPC],
            "wlr": wlr,
        }
        for i in range(N_CORES)
    ]
    trace = bool(int(os.environ.get("BASS_KERNEL_TRACE", "0")))
    kw = {}
    if os.environ.get("BASS_KERNEL_TMPDIR"):
        kw["tmpdir"] = os.environ["BASS_KERNEL_TMPDIR"]
    res = run_bass_kernel_spmd(nc, in_maps, list(range(N_CORES)), trace=trace, **kw)
    _PROGRAM_CACHE["last_results"] = res

    # Assemble A0/C0 [B, T]: dots for core-local batches 0..2, host dots for
    # batch 3's raw segment sums.
    A0 = np.empty((B, T), np.float64)
    C0 = np.empty((B, T), np.float64)
    for i, r in enumerate(res.results):
        dots = r["dots"]  # [T, BPC-1, 2]
        seg3 = r["seg3"].astype(np.float64)  # [T, D]
        for j in range(BPC - 1):
            A0[i * BPC + j] = dots[:, j, 0]
            C0[i * BPC + j] = dots[:, j, 1]
        A0[i * BPC + BPC - 1] = seg3 @ wlr[0].astype(np.float64)
        C0[i * BPC + BPC - 1] = seg3 @ wlr[1].astype(np.float64)
    return _host_finish(A0, C0, counts, bd)
